# revision 1
# baseline (speedup 1.0000x reference)
"""ConvCaps (nn_ConvCaps_34995393528409) Trainium2 Bass kernel.

Math: out[b,h,w,x,y,o,m,n] = sum_i poses[b,h+x,w+y,i,m,n] * kernel[x,y,i,o,m,n]

Strategy ("Z-trick"):
  Z[b,p,x,y,o,m,n] = sum_i poses[b,p,i,m,n] * kernel[x,y,i,o,m,n]   (p = every
  input position). Then out[b,h,w,...] = Z[b,(h+x,w+y),x,y,...] is a pure
  re-indexing done by shifted-rectangle DMA writes, and the HBM layout of each
  output position's (x,y,o,m,n) block (4608 f32) is contiguous.

Per core (batch-sharded, B_LOCAL=4):
  - SBUF partition dim = q = b*20 + pw  (80 of 128) so each (b,x,y) scatter is
    a single contiguous partition range [b*20+y, +18).
  - Matmul per (ph, mn): stationary = poses [K=32 i, M=80 q], moving =
    kernel [K=32 i, N<=288 (x,y,o)], fp32, 4x PE row-tiling over mn groups
    (tile_position=(32r,0), mn = r*4+j).
  - PSUM [80, 288] evacuated by DVE/ACT with a stride-16 interleave so each
    partition's free dim becomes the HBM-contiguous (x,y,o,m,n) block.
  - 9 scatter DMAs per 4-row band write 2KB-contiguous runs straight into the
    output tensor.
"""

import os

import numpy as np

import concourse.bass as bass
import concourse.tile as tile
from concourse import bacc, mybir
from concourse.vector_clock import ScopedClock

F32 = mybir.dt.float32

N_CORES = 8
B_LOCAL = 4  # 32 / 8
HW20 = 20
OHW = 18
NI = 32
NO = 32
NMN = 16
NXY = 9
XYO = NXY * NO  # 288
BLK = XYO * NMN  # 4608 floats per (b,p) output block
PH_PER_BAND = 4
N_BANDS = 5
NQ = HW20 * B_LOCAL  # 80 partitions used
ZROW = PH_PER_BAND * BLK  # z free floats per partition per band

# output strides (elements) for the full per-core output [4,18,18,9,512]
OS_B = OHW * OHW * NXY * 512  # 1492992
OS_H = OHW * NXY * 512  # 82944
OS_W = NXY * 512  # 4608


def _patch_tile_drain():
    """This walrus build rejects >1 sync-wait on the Tile kernel-tail Drain;
    split the waits across a chain of drains."""
    if getattr(tile.TileContext, "_convcaps_drain_patch", False):
        return

    def _drain_and_barrier(self, tick_clock, wait_clock):
        drain_inst = self.nc.sync.drain()
        wait_clock.add_sem_waits(
            drain_inst.ins, ScopedClock({None: tick_clock.global_clock})
        )
        si = drain_inst.ins.sync_info
        w = list(si.on_wait or []) if si is not None else []
        if len(w) > 1:
            drain_inst.ins.sync_info = mybir.SyncInfo(
                on_wait=w[:1], on_update=list(si.on_update or [])
            )
            for x in w[1:]:
                extra = self.nc.sync.drain()
                extra.ins.sync_info = mybir.SyncInfo(on_wait=[x], on_update=[])
        self.nc.all_engine_barrier()
        assert self.sems is not None
        popped = self.nc._tile_sem_poison_stack.pop()
        assert popped is self._sem_poison
        self.nc.clear_and_free_semaphores(list(self.sems.allocated().values()))
        self.nc.all_engine_barrier()

    tile.TileContext._drain_and_barrier = _drain_and_barrier
    tile.TileContext._convcaps_drain_patch = True


def _build_nc(n_bands=N_BANDS):
    _patch_tile_drain()
    use_f32r = os.environ.get("CONVCAPS_F32R", "0") == "1"
    IN_DT = mybir.dt.float32r if use_f32r else F32
    nc = bacc.Bacc("TRN2", target_bir_lowering=False, num_devices=N_CORES)

    # [r, i, (j, ph, q)] ; free idx = j*1600 + ph*80 + q ; q = b*20 + pw
    poses_d = nc.declare_dram_parameter(
        "poses_t", [4, NI, 4 * HW20 * NQ], IN_DT, isOutput=False
    )
    # [r, i, (j, xy, o)] ; free idx = j*288 + xy*32 + o
    kern_d = nc.declare_dram_parameter(
        "kern_t", [4, NI, 4 * XYO], IN_DT, isOutput=False
    )
    out_d = nc.declare_dram_parameter(
        "out", [B_LOCAL, OHW, OHW, NXY, 512], F32, isOutput=True
    )

    pose_sb = nc.alloc_sbuf_tensor("pose_sb", [128, 4 * HW20 * NQ], IN_DT)
    kern_sb = nc.alloc_sbuf_tensor("kern_sb", [128, 4 * XYO], IN_DT)
    z_sb = [nc.alloc_sbuf_tensor(f"z{i}", [128, ZROW], F32) for i in range(2)]

    pose_f = 4 * HW20 * NQ  # pose_sb free width
    kern_f = 4 * XYO

    with tile.TileContext(nc) as tc:
        for r in range(4):
            nc.sync.dma_start(
                pose_sb.ap()[32 * r : 32 * r + 32, :], poses_d.ap()[r]
            )
            nc.sync.dma_start(
                kern_sb.ap()[32 * r : 32 * r + 32, :], kern_d.ap()[r]
            )

        with tc.tile_pool(name="psum", bufs=8, space="PSUM") as pp:
            for band in range(n_bands):
                z = z_sb[band % 2]
                for phl in range(PH_PER_BAND):
                    ph = band * PH_PER_BAND + phl
                    x0 = max(0, ph - (OHW - 1))
                    x1 = min(2, ph)
                    n_xyo = (x1 - x0 + 1) * 3 * NO  # valid (x,y,o) span
                    for j in range(4):
                        for r in range(4):
                            mn = r * 4 + j
                            ps = pp.tile([128, XYO], F32, name="ps", tag="ps")
                            nc.tensor.matmul(
                                ps[0:NQ, 0:n_xyo],
                                pose_sb.ap()[
                                    32 * r : 32 * r + 32,
                                    j * (HW20 * NQ) + ph * NQ : j * (HW20 * NQ)
                                    + ph * NQ
                                    + NQ,
                                ],
                                kern_sb.ap()[
                                    32 * r : 32 * r + 32,
                                    j * XYO + x0 * 96 : j * XYO + (x1 + 1) * 96,
                                ],
                                start=True,
                                stop=True,
                                tile_position=(32 * r, 0),
                            )
                            dst = bass.AP(
                                z,
                                phl * BLK + x0 * 96 * NMN + mn,
                                [[ZROW, NQ], [NMN, n_xyo]],
                            )
                            if r == 2 or (r == 3 and j % 2 == 0):
                                nc.scalar.copy(dst, ps[0:NQ, 0:n_xyo])
                            else:
                                nc.vector.tensor_copy(dst, ps[0:NQ, 0:n_xyo])

                # scatter this band into the output
                for x in range(3):
                    ph0 = max(x, band * PH_PER_BAND)
                    ph_end = min(x + OHW, band * PH_PER_BAND + PH_PER_BAND)
                    nh = ph_end - ph0
                    if nh <= 0:
                        continue
                    phl0 = ph0 - band * PH_PER_BAND
                    for y in range(3):
                        xy = x * 3 + y
                        for b in range(B_LOCAL):
                            src = bass.AP(
                                z,
                                (b * HW20 + y) * ZROW + phl0 * BLK + xy * 512,
                                [[ZROW, OHW], [BLK, nh], [1, 512]],
                            )
                            dst = bass.AP(
                                out_d,
                                b * OS_B + (ph0 - x) * OS_H + xy * 512,
                                [[OS_W, OHW], [OS_H, nh], [1, 512]],
                            )
                            eng = nc.sync if (xy + b) % 2 == 0 else nc.gpsimd
                            eng.dma_start(dst, src)
    nc.finalize()
    return nc


_NC_CACHE = None


def _get_nc():
    global _NC_CACHE
    if _NC_CACHE is None:
        _NC_CACHE = _build_nc(
            int(os.environ.get("CONVCAPS_BANDS", str(N_BANDS)))
        )
    return _NC_CACHE


def _prep_poses(shard: np.ndarray) -> np.ndarray:
    # shard: (4, 20, 20, 32, 4, 4) -> [r, i, (j, ph, b*20+pw)]
    a = shard.reshape(B_LOCAL, HW20, HW20, NI, NMN)
    a = a.transpose(4, 3, 1, 0, 2)  # [mn, i, ph, b, pw]
    a = a.reshape(4, 4, NI, HW20, B_LOCAL, HW20)  # [r, j, i, ph, b, pw]
    a = a.transpose(0, 2, 1, 3, 4, 5)  # [r, i, j, ph, b, pw]
    return np.ascontiguousarray(a.reshape(4, NI, 4 * HW20 * NQ), dtype=np.float32)


def _prep_kernel(kern: np.ndarray) -> np.ndarray:
    # kern: (3, 3, 32, 32, 4, 4) -> [r, i, (j, xy, o)]
    a = kern.reshape(3, 3, NI, NO, NMN)
    a = a.transpose(4, 2, 0, 1, 3)  # [mn, i, x, y, o]
    a = a.reshape(4, 4, NI, 3, 3, NO)  # [r, j, i, x, y, o]
    a = a.transpose(0, 2, 1, 3, 4, 5)  # [r, i, j, x, y, o]
    return np.ascontiguousarray(a.reshape(4, NI, 4 * XYO), dtype=np.float32)


LAST_RESULTS = None  # set when CONVCAPS_TRACE=1, for test harness introspection


def kernel(**inputs) -> np.ndarray:
    from concourse.bass_utils import run_bass_kernel_spmd

    poses = np.asarray(inputs["poses"], dtype=np.float32)
    kern = np.asarray(inputs["kernel"], dtype=np.float32)

    nc = _get_nc()
    kern_t = _prep_kernel(kern)
    in_maps = []
    for c in range(N_CORES):
        shard = poses[c * B_LOCAL : (c + 1) * B_LOCAL]
        in_maps.append({"poses_t": _prep_poses(shard), "kern_t": kern_t})

    trace = os.environ.get("CONVCAPS_TRACE", "0") == "1"
    res = run_bass_kernel_spmd(
        nc, in_maps, core_ids=list(range(N_CORES)), trace=trace
    )
    if trace:
        global LAST_RESULTS
        LAST_RESULTS = res

    out = np.concatenate(
        [
            r["out"].reshape(B_LOCAL, OHW, OHW, 3, 3, NO, 4, 4)
            for r in res.results
        ],
        axis=0,
    )
    return out



# revision 10
# speedup vs baseline: 1.1429x; 1.1429x over previous
"""ConvCaps (nn_ConvCaps_34995393528409) Trainium2 Bass kernel, v2.

Math: out[b,h,w,x,y,o,m,n] = sum_i poses[b,h+x,w+y,i,m,n] * kernel[x,y,i,o,m,n]

Strategy ("position-major"):
  PSUM/output partition = flat output position c = (h-h0)*20 + w for an
  (b, h-band) tile, so the staging buffer z[c, (xy,o,m,n)] is HBM-ordered and
  the final DMA writes 18KB-contiguous runs (vs 2KB scatter in v1), engaging
  all 16 DMA engines near line rate.

  Per (b, h-band, xy): one PSUM bank holds the complete output-ordered 512-f32
  block (o, m, n). It is filled by 4 matmuls (one per m=j quartet):
    K = (n-member mnA, i) = 128 (block-diagonal weights: W''[(mnA,i),(o,mnB)]
        = kernel[x,y,i,o,(j,mnB)] * delta(mnA,mnB))
    lhsT (stationary) = poses patch [128, M=128 flat positions] -- the (x,y)
        shift is a pure AP offset into PJ[(mnA,i), (b,ph,pw)]
    N = 128 cols (o, mnB) written strided [[16,32],[1,4]] at offset j*4 so the
        bank accumulates (o, m, n) in final order.
  Evacuation is then a contiguous [128, 512] copy per bank (DVE/ACT at full
  rate instead of stride-16 interleave), and per tile one dma_start moves
  z -> out with 108 descriptors of 18.4KB.

  bf16 inputs (fp32 accumulate) halve input DMA and enable FWL weight loads;
  CONVCAPS_IN_DT=f32 falls back to fp32 inputs.
"""

import os

import numpy as np

import concourse.bass as bass
import concourse.tile as tile
from concourse import bacc, mybir
from concourse.vector_clock import ScopedClock

F32 = mybir.dt.float32
BF16 = mybir.dt.bfloat16

N_CORES = 8
B_LOCAL = 4  # 32 / 8
HW20 = 20
OHW = 18
NI = 32
NO = 32
NXY = 9
NH = 6  # h rows per tile
N_HTILES = OHW // NH  # 3
NPOS = 128  # flat positions (partitions) per tile: c = h'*20 + pw, real c%20<18
PJ_F = 1632  # padded free size per j block: >= 3*400 + 14*20 + 2 + 128
BLK = 512  # (o, m, n) floats per (position, xy)
ZROW = NXY * BLK  # 4608 floats per position

# output strides (elements) for the per-core output [4,18,18,9,512]
OS_B = OHW * OHW * NXY * BLK  # 1492992
OS_H = OHW * NXY * BLK  # 82944
OS_W = NXY * BLK  # 4608


def _patch_tile_drain():
    """This walrus build rejects >1 sync-wait on the Tile kernel-tail Drain;
    split the waits across a chain of drains."""
    if getattr(tile.TileContext, "_convcaps_drain_patch", False):
        return

    def _drain_and_barrier(self, tick_clock, wait_clock):
        drain_inst = self.nc.sync.drain()
        wait_clock.add_sem_waits(
            drain_inst.ins, ScopedClock({None: tick_clock.global_clock})
        )
        si = drain_inst.ins.sync_info
        w = list(si.on_wait or []) if si is not None else []
        if len(w) > 1:
            drain_inst.ins.sync_info = mybir.SyncInfo(
                on_wait=w[:1], on_update=list(si.on_update or [])
            )
            for x in w[1:]:
                extra = self.nc.sync.drain()
                extra.ins.sync_info = mybir.SyncInfo(on_wait=[x], on_update=[])
        self.nc.all_engine_barrier()
        assert self.sems is not None
        popped = self.nc._tile_sem_poison_stack.pop()
        assert popped is self._sem_poison
        self.nc.clear_and_free_semaphores(list(self.sems.allocated().values()))
        self.nc.all_engine_barrier()

    tile.TileContext._drain_and_barrier = _drain_and_barrier
    tile.TileContext._convcaps_drain_patch = True


def _in_dt():
    return F32 if os.environ.get("CONVCAPS_IN_DT", "bf16") == "f32" else BF16


def _build_nc():
    _patch_tile_drain()
    IN_DT = _in_dt()
    nc = bacc.Bacc("TRN2", target_bir_lowering=False, num_devices=N_CORES)

    # [j, (mnA, i), (b, ph, pw)] zero-padded to PJ_F
    pj_d = nc.declare_dram_parameter("pj_t", [4, 128, PJ_F], IN_DT, isOutput=False)
    # [(mnA, i), (xy, j, o, mnB)]
    ww_d = nc.declare_dram_parameter("ww_t", [128, NXY * 4 * 128], IN_DT, isOutput=False)
    out_d = nc.declare_dram_parameter(
        "out", [B_LOCAL, OHW, OHW, NXY, BLK], F32, isOutput=True
    )

    pj_sb = nc.alloc_sbuf_tensor("pj_sb", [128, 4 * PJ_F], IN_DT)
    ww_sb = nc.alloc_sbuf_tensor("ww_sb", [128, NXY * 4 * 128], IN_DT)
    z_sb = [nc.alloc_sbuf_tensor(f"z{i}", [128, ZROW], F32) for i in range(3)]

    with tile.TileContext(nc) as tc:
        for j in range(4):
            nc.sync.dma_start(
                pj_sb.ap()[0:128, j * PJ_F : (j + 1) * PJ_F], pj_d.ap()[j]
            )
        nc.sync.dma_start(ww_sb.ap()[0:128, :], ww_d.ap()[0:128, :])

        with tc.tile_pool(name="psum", bufs=8, space="PSUM") as pp:
            t_idx = 0
            for b in range(B_LOCAL):
                for ht in range(N_HTILES):
                    h0 = ht * NH
                    z = z_sb[t_idx % 3]
                    for xy in range(NXY):
                        x, y = xy // 3, xy % 3
                        ps = pp.tile([128, BLK], F32, name="ps", tag="ps")
                        for j in range(4):
                            base = j * PJ_F + b * 400 + (h0 + x) * HW20 + y
                            dst = bass.AP(
                                ps.tensor,
                                j * 4,
                                [[BLK, NPOS], [16, NO], [1, 4]],
                            )
                            nc.tensor.matmul(
                                dst,
                                pj_sb.ap()[0:128, base : base + 128],
                                ww_sb.ap()[
                                    0:128, xy * 512 + j * 128 : xy * 512 + (j + 1) * 128
                                ],
                                start=(j == 0),
                                stop=(j == 3),
                                skip_group_check=True,
                            )
                        zdst = z.ap()[0:NPOS, xy * BLK : (xy + 1) * BLK]
                        if xy % 2 == 0:
                            nc.vector.tensor_copy(zdst, ps[0:NPOS, 0:BLK])
                        else:
                            nc.scalar.copy(zdst, ps[0:NPOS, 0:BLK])
                    # per h row: z[c=h'*20+w (18 partitions), 4608] ->
                    # out[b, h0+h', :, :, :] (one contiguous 331KB HBM range)
                    for hp in range(NH):
                        src = bass.AP(
                            z, hp * HW20 * ZROW, [[ZROW, OHW], [1, ZROW]]
                        )
                        dst = bass.AP(
                            out_d,
                            b * OS_B + (h0 + hp) * OS_H,
                            [[ZROW, OHW], [1, ZROW]],
                        )
                        nc.sync.dma_start(dst, src)
                    t_idx += 1
    nc.finalize()
    return nc


_NC_CACHE = None


def _get_nc():
    global _NC_CACHE
    if _NC_CACHE is None:
        _NC_CACHE = _build_nc()
    return _NC_CACHE


def _np_in_dt():
    if _in_dt() == F32:
        return np.float32
    import ml_dtypes

    return ml_dtypes.bfloat16


def _prep_pj(shard: np.ndarray) -> np.ndarray:
    # shard: (4, 20, 20, 32, 4, 4) -> [j, (mnA, i), (b, ph, pw)] padded
    a = shard.reshape(B_LOCAL, HW20, HW20, NI, 4, 4)  # b, ph, pw, i, j(=m), mnA(=n)
    a = a.transpose(4, 5, 3, 0, 1, 2)  # j, mnA, i, b, ph, pw
    a = a.reshape(4, 128, B_LOCAL * HW20 * HW20)
    out = np.zeros((4, 128, PJ_F), dtype=np.float32)
    out[:, :, : B_LOCAL * HW20 * HW20] = a
    return np.ascontiguousarray(out).astype(_np_in_dt())


def _prep_ww(kern: np.ndarray) -> np.ndarray:
    # kern: (3, 3, 32, 32, 4, 4) -> [(mnA, i), (xy, j, o, mnB)] block-diagonal
    k9 = kern.reshape(NXY, NI, NO, 4, 4)  # xy, i, o, j(=m), mnB(=n)
    ww = np.zeros((4, NI, NXY, 4, NO, 4), dtype=np.float32)  # mnA,i,xy,j,o,mnB
    for d in range(4):
        ww[d, :, :, :, :, d] = k9[:, :, :, :, d].transpose(1, 0, 3, 2)  # i,xy,j,o
    return np.ascontiguousarray(ww.reshape(128, NXY * 4 * 128)).astype(_np_in_dt())


LAST_RESULTS = None  # set when CONVCAPS_TRACE=1, for test harness introspection


def kernel(**inputs) -> np.ndarray:
    from concourse.bass_utils import run_bass_kernel_spmd

    poses = np.asarray(inputs["poses"], dtype=np.float32)
    kern = np.asarray(inputs["kernel"], dtype=np.float32)

    nc = _get_nc()
    ww_t = _prep_ww(kern)
    in_maps = []
    for c in range(N_CORES):
        shard = poses[c * B_LOCAL : (c + 1) * B_LOCAL]
        in_maps.append({"pj_t": _prep_pj(shard), "ww_t": ww_t})

    trace = os.environ.get("CONVCAPS_TRACE", "0") == "1"
    res = run_bass_kernel_spmd(
        nc, in_maps, core_ids=list(range(N_CORES)), trace=trace
    )
    if trace:
        global LAST_RESULTS
        LAST_RESULTS = res

    out = np.concatenate(
        [
            r["out"].reshape(B_LOCAL, OHW, OHW, 3, 3, NO, 4, 4)
            for r in res.results
        ],
        axis=0,
    )
    return out


# revision 11
# speedup vs baseline: 1.1572x; 1.0126x over previous
"""ConvCaps (nn_ConvCaps_34995393528409) Trainium2 Bass kernel, v2.

Math: out[b,h,w,x,y,o,m,n] = sum_i poses[b,h+x,w+y,i,m,n] * kernel[x,y,i,o,m,n]

Strategy ("position-major"):
  PSUM/output partition = flat output position c = (h-h0)*20 + w for an
  (b, h-band) tile, so the staging buffer z[c, (xy,o,m,n)] is HBM-ordered and
  the final DMA writes 18KB-contiguous runs (vs 2KB scatter in v1), engaging
  all 16 DMA engines near line rate.

  Per (b, h-band, xy): one PSUM bank holds the complete output-ordered 512-f32
  block (o, m, n). It is filled by 4 matmuls (one per m=j quartet):
    K = (n-member mnA, i) = 128 (block-diagonal weights: W''[(mnA,i),(o,mnB)]
        = kernel[x,y,i,o,(j,mnB)] * delta(mnA,mnB))
    lhsT (stationary) = poses patch [128, M=128 flat positions] -- the (x,y)
        shift is a pure AP offset into PJ[(mnA,i), (b,ph,pw)]
    N = 128 cols (o, mnB) written strided [[16,32],[1,4]] at offset j*4 so the
        bank accumulates (o, m, n) in final order.
  Evacuation is then a contiguous [128, 512] copy per bank (DVE/ACT at full
  rate instead of stride-16 interleave), and per tile one dma_start moves
  z -> out with 108 descriptors of 18.4KB.

  bf16 inputs (fp32 accumulate) halve input DMA and enable FWL weight loads;
  CONVCAPS_IN_DT=f32 falls back to fp32 inputs.
"""

import os

import numpy as np

import concourse.bass as bass
import concourse.tile as tile
from concourse import bacc, mybir
from concourse.vector_clock import ScopedClock

F32 = mybir.dt.float32
BF16 = mybir.dt.bfloat16

N_CORES = 8
B_LOCAL = 4  # 32 / 8
HW20 = 20
OHW = 18
NI = 32
NO = 32
NXY = 9
NH = 6  # h rows per tile
N_HTILES = OHW // NH  # 3
NPOS = 128  # flat positions (partitions) per tile: c = h'*20 + pw, real c%20<18
PJ_F = 1632  # padded free size per j block: >= 3*400 + 14*20 + 2 + 128
BLK = 512  # (o, m, n) floats per (position, xy)
ZROW = NXY * BLK  # 4608 floats per position

# output strides (elements) for the per-core output [4,18,18,9,512]
OS_B = OHW * OHW * NXY * BLK  # 1492992
OS_H = OHW * NXY * BLK  # 82944
OS_W = NXY * BLK  # 4608


def _patch_tile_drain():
    """This walrus build rejects >1 sync-wait on the Tile kernel-tail Drain;
    split the waits across a chain of drains."""
    if getattr(tile.TileContext, "_convcaps_drain_patch", False):
        return

    def _drain_and_barrier(self, tick_clock, wait_clock):
        drain_inst = self.nc.sync.drain()
        wait_clock.add_sem_waits(
            drain_inst.ins, ScopedClock({None: tick_clock.global_clock})
        )
        si = drain_inst.ins.sync_info
        w = list(si.on_wait or []) if si is not None else []
        if len(w) > 1:
            drain_inst.ins.sync_info = mybir.SyncInfo(
                on_wait=w[:1], on_update=list(si.on_update or [])
            )
            for x in w[1:]:
                extra = self.nc.sync.drain()
                extra.ins.sync_info = mybir.SyncInfo(on_wait=[x], on_update=[])
        self.nc.all_engine_barrier()
        assert self.sems is not None
        popped = self.nc._tile_sem_poison_stack.pop()
        assert popped is self._sem_poison
        self.nc.clear_and_free_semaphores(list(self.sems.allocated().values()))
        self.nc.all_engine_barrier()

    tile.TileContext._drain_and_barrier = _drain_and_barrier
    tile.TileContext._convcaps_drain_patch = True


def _in_dt():
    return F32 if os.environ.get("CONVCAPS_IN_DT", "bf16") == "f32" else BF16


def _build_nc():
    _patch_tile_drain()
    IN_DT = _in_dt()
    nc = bacc.Bacc("TRN2", target_bir_lowering=False, num_devices=N_CORES)

    # [j, (mnA, i), (b, ph, pw)] zero-padded to PJ_F
    pj_d = nc.declare_dram_parameter("pj_t", [4, 128, PJ_F], IN_DT, isOutput=False)
    # [(mnA, i), (xy, j, o, mnB)]
    ww_d = nc.declare_dram_parameter("ww_t", [128, NXY * 4 * 128], IN_DT, isOutput=False)
    out_d = nc.declare_dram_parameter(
        "out", [B_LOCAL, OHW, OHW, NXY, BLK], F32, isOutput=True
    )

    pj_sb = nc.alloc_sbuf_tensor("pj_sb", [128, 4 * PJ_F], IN_DT)
    ww_sb = nc.alloc_sbuf_tensor("ww_sb", [128, NXY * 4 * 128], IN_DT)
    z_sb = [nc.alloc_sbuf_tensor(f"z{i}", [128, ZROW], F32) for i in range(3)]

    with tile.TileContext(nc) as tc:
        for j in range(4):
            nc.sync.dma_start(
                pj_sb.ap()[0:128, j * PJ_F : (j + 1) * PJ_F], pj_d.ap()[j]
            )
        nc.sync.dma_start(ww_sb.ap()[0:128, :], ww_d.ap()[0:128, :])

        with tc.tile_pool(name="psum", bufs=8, space="PSUM") as pp:
            t_idx = 0
            for b in range(B_LOCAL):
                for ht in range(N_HTILES):
                    h0 = ht * NH
                    z = z_sb[t_idx % 3]
                    for xy in range(NXY):
                        x, y = xy // 3, xy % 3
                        ps = pp.tile([128, BLK], F32, name="ps", tag="ps")
                        for j in range(4):
                            base = j * PJ_F + b * 400 + (h0 + x) * HW20 + y
                            dst = bass.AP(
                                ps.tensor,
                                j * 4,
                                [[BLK, NPOS], [16, NO], [1, 4]],
                            )
                            nc.tensor.matmul(
                                dst,
                                pj_sb.ap()[0:128, base : base + 128],
                                ww_sb.ap()[
                                    0:128, xy * 512 + j * 128 : xy * 512 + (j + 1) * 128
                                ],
                                start=(j == 0),
                                stop=(j == 3),
                                skip_group_check=True,
                            )
                        zdst = z.ap()[0:NPOS, xy * BLK : (xy + 1) * BLK]
                        if xy % 2 == 0:
                            nc.vector.tensor_copy(zdst, ps[0:NPOS, 0:BLK])
                        else:
                            nc.scalar.copy(zdst, ps[0:NPOS, 0:BLK])
                    # per h row: z[c=h'*20+w (18 partitions), 4608] ->
                    # out[b, h0+h', :, :, :] (one contiguous 331KB HBM range).
                    # Spread across queues: sync/scalar HWDGE feed SDMA 0-8,
                    # gpsimd SWDGE feeds 9-15 -> weight gpsimd ~half the bytes.
                    dma_engs = [
                        nc.gpsimd, nc.sync, nc.gpsimd, nc.scalar, nc.gpsimd, nc.sync
                    ]
                    for hp in range(NH):
                        src = bass.AP(
                            z, hp * HW20 * ZROW, [[ZROW, OHW], [1, ZROW]]
                        )
                        dst = bass.AP(
                            out_d,
                            b * OS_B + (h0 + hp) * OS_H,
                            [[ZROW, OHW], [1, ZROW]],
                        )
                        dma_engs[hp].dma_start(dst, src)
                    t_idx += 1
    nc.finalize()
    return nc


_NC_CACHE = None


def _get_nc():
    global _NC_CACHE
    if _NC_CACHE is None:
        _NC_CACHE = _build_nc()
    return _NC_CACHE


def _np_in_dt():
    if _in_dt() == F32:
        return np.float32
    import ml_dtypes

    return ml_dtypes.bfloat16


def _prep_pj(shard: np.ndarray) -> np.ndarray:
    # shard: (4, 20, 20, 32, 4, 4) -> [j, (mnA, i), (b, ph, pw)] padded
    a = shard.reshape(B_LOCAL, HW20, HW20, NI, 4, 4)  # b, ph, pw, i, j(=m), mnA(=n)
    a = a.transpose(4, 5, 3, 0, 1, 2)  # j, mnA, i, b, ph, pw
    a = a.reshape(4, 128, B_LOCAL * HW20 * HW20)
    out = np.zeros((4, 128, PJ_F), dtype=np.float32)
    out[:, :, : B_LOCAL * HW20 * HW20] = a
    return np.ascontiguousarray(out).astype(_np_in_dt())


def _prep_ww(kern: np.ndarray) -> np.ndarray:
    # kern: (3, 3, 32, 32, 4, 4) -> [(mnA, i), (xy, j, o, mnB)] block-diagonal
    k9 = kern.reshape(NXY, NI, NO, 4, 4)  # xy, i, o, j(=m), mnB(=n)
    ww = np.zeros((4, NI, NXY, 4, NO, 4), dtype=np.float32)  # mnA,i,xy,j,o,mnB
    for d in range(4):
        ww[d, :, :, :, :, d] = k9[:, :, :, :, d].transpose(1, 0, 3, 2)  # i,xy,j,o
    return np.ascontiguousarray(ww.reshape(128, NXY * 4 * 128)).astype(_np_in_dt())


LAST_RESULTS = None  # set when CONVCAPS_TRACE=1, for test harness introspection


def kernel(**inputs) -> np.ndarray:
    from concourse.bass_utils import run_bass_kernel_spmd

    poses = np.asarray(inputs["poses"], dtype=np.float32)
    kern = np.asarray(inputs["kernel"], dtype=np.float32)

    nc = _get_nc()
    ww_t = _prep_ww(kern)
    in_maps = []
    for c in range(N_CORES):
        shard = poses[c * B_LOCAL : (c + 1) * B_LOCAL]
        in_maps.append({"pj_t": _prep_pj(shard), "ww_t": ww_t})

    trace = os.environ.get("CONVCAPS_TRACE", "0") == "1"
    res = run_bass_kernel_spmd(
        nc, in_maps, core_ids=list(range(N_CORES)), trace=trace
    )
    if trace:
        global LAST_RESULTS
        LAST_RESULTS = res

    out = np.concatenate(
        [
            r["out"].reshape(B_LOCAL, OHW, OHW, 3, 3, NO, 4, 4)
            for r in res.results
        ],
        axis=0,
    )
    return out


# revision 14
# speedup vs baseline: 1.1702x; 1.0112x over previous
"""ConvCaps (nn_ConvCaps_34995393528409) Trainium2 Bass kernel, v2.

Math: out[b,h,w,x,y,o,m,n] = sum_i poses[b,h+x,w+y,i,m,n] * kernel[x,y,i,o,m,n]

Strategy ("position-major"):
  PSUM/output partition = flat output position c = (h-h0)*20 + w for an
  (b, h-band) tile, so the staging buffer z[c, (xy,o,m,n)] is HBM-ordered and
  the final DMA writes 18KB-contiguous runs (vs 2KB scatter in v1), engaging
  all 16 DMA engines near line rate.

  Per (b, h-band, xy): one PSUM bank holds the complete output-ordered 512-f32
  block (o, m, n). It is filled by 4 matmuls (one per m=j quartet):
    K = (n-member mnA, i) = 128 (block-diagonal weights: W''[(mnA,i),(o,mnB)]
        = kernel[x,y,i,o,(j,mnB)] * delta(mnA,mnB))
    lhsT (stationary) = poses patch [128, M=128 flat positions] -- the (x,y)
        shift is a pure AP offset into PJ[(mnA,i), (b,ph,pw)]
    N = 128 cols (o, mnB) written strided [[16,32],[1,4]] at offset j*4 so the
        bank accumulates (o, m, n) in final order.
  Evacuation is then a contiguous [128, 512] copy per bank (DVE/ACT at full
  rate instead of stride-16 interleave), and per tile one dma_start moves
  z -> out with 108 descriptors of 18.4KB.

  bf16 inputs (fp32 accumulate) halve input DMA and enable FWL weight loads;
  CONVCAPS_IN_DT=f32 falls back to fp32 inputs.
"""

import os

import numpy as np

import concourse.bass as bass
import concourse.tile as tile
from concourse import bacc, mybir
from concourse.vector_clock import ScopedClock

F32 = mybir.dt.float32
BF16 = mybir.dt.bfloat16

N_CORES = 8
B_LOCAL = 4  # 32 / 8
HW20 = 20
OHW = 18
NI = 32
NO = 32
NXY = 9
NH = 6  # h rows per tile
N_HTILES = OHW // NH  # 3
NPOS = 128  # flat positions (partitions) per tile: c = h'*20 + pw, real c%20<18
PJ_F = 1632  # padded free size per j block: >= 3*400 + 14*20 + 2 + 128
BLK = 512  # (o, m, n) floats per (position, xy)
ZROW = NXY * BLK  # 4608 floats per position

# output strides (elements) for the per-core output [4,18,18,9,512]
OS_B = OHW * OHW * NXY * BLK  # 1492992
OS_H = OHW * NXY * BLK  # 82944
OS_W = NXY * BLK  # 4608


def _patch_tile_drain():
    """This walrus build rejects >1 sync-wait on the Tile kernel-tail Drain;
    split the waits across a chain of drains."""
    if getattr(tile.TileContext, "_convcaps_drain_patch", False):
        return

    def _drain_and_barrier(self, tick_clock, wait_clock):
        drain_inst = self.nc.sync.drain()
        wait_clock.add_sem_waits(
            drain_inst.ins, ScopedClock({None: tick_clock.global_clock})
        )
        si = drain_inst.ins.sync_info
        w = list(si.on_wait or []) if si is not None else []
        if len(w) > 1:
            drain_inst.ins.sync_info = mybir.SyncInfo(
                on_wait=w[:1], on_update=list(si.on_update or [])
            )
            for x in w[1:]:
                extra = self.nc.sync.drain()
                extra.ins.sync_info = mybir.SyncInfo(on_wait=[x], on_update=[])
        self.nc.all_engine_barrier()
        assert self.sems is not None
        popped = self.nc._tile_sem_poison_stack.pop()
        assert popped is self._sem_poison
        self.nc.clear_and_free_semaphores(list(self.sems.allocated().values()))
        self.nc.all_engine_barrier()

    tile.TileContext._drain_and_barrier = _drain_and_barrier
    tile.TileContext._convcaps_drain_patch = True


def _in_dt():
    return F32 if os.environ.get("CONVCAPS_IN_DT", "bf16") == "f32" else BF16


def _build_nc():
    _patch_tile_drain()
    IN_DT = _in_dt()
    nc = bacc.Bacc("TRN2", target_bir_lowering=False, num_devices=N_CORES)

    # [j, (mnA, i), (b, ph, pw)] zero-padded to PJ_F
    pj_d = nc.declare_dram_parameter("pj_t", [4, 128, PJ_F], IN_DT, isOutput=False)
    # [(mnA, i), (xy, j, o, mnB)]
    ww_d = nc.declare_dram_parameter("ww_t", [128, NXY * 4 * 128], IN_DT, isOutput=False)
    out_d = nc.declare_dram_parameter(
        "out", [B_LOCAL, OHW, OHW, NXY, BLK], F32, isOutput=True
    )

    pj_sb = nc.alloc_sbuf_tensor("pj_sb", [128, 4 * PJ_F], IN_DT)
    ww_sb = nc.alloc_sbuf_tensor("ww_sb", [128, NXY * 4 * 128], IN_DT)
    z_sb = [nc.alloc_sbuf_tensor(f"z{i}", [128, ZROW], F32) for i in range(4)]

    with tile.TileContext(nc) as tc:
        nc.sync.dma_start(ww_sb.ap()[0:128, :], ww_d.ap()[0:128, :])
        for j in range(4):
            eng = nc.sync if j % 2 == 0 else nc.scalar
            eng.dma_start(
                pj_sb.ap()[0:128, j * PJ_F : (j + 1) * PJ_F], pj_d.ap()[j]
            )

        with tc.tile_pool(name="psum", bufs=8, space="PSUM") as pp:
            t_idx = 0
            for b in range(B_LOCAL):
                for ht in range(N_HTILES):
                    h0 = ht * NH
                    z = z_sb[t_idx % 4]
                    for xy in range(NXY):
                        x, y = xy // 3, xy % 3
                        ps = pp.tile([128, BLK], F32, name="ps", tag="ps")
                        for j in range(4):
                            base = j * PJ_F + b * 400 + (h0 + x) * HW20 + y
                            dst = bass.AP(
                                ps.tensor,
                                j * 4,
                                [[BLK, NPOS], [16, NO], [1, 4]],
                            )
                            nc.tensor.matmul(
                                dst,
                                pj_sb.ap()[0:128, base : base + 128],
                                ww_sb.ap()[
                                    0:128, xy * 512 + j * 128 : xy * 512 + (j + 1) * 128
                                ],
                                start=(j == 0),
                                stop=(j == 3),
                                skip_group_check=True,
                            )
                        zdst = z.ap()[0:NPOS, xy * BLK : (xy + 1) * BLK]
                        if xy % 2 == 0:
                            nc.vector.tensor_copy(zdst, ps[0:NPOS, 0:BLK])
                        else:
                            nc.scalar.copy(zdst, ps[0:NPOS, 0:BLK])
                    # per h row: z[c=h'*20+w (18 partitions), 4608] ->
                    # out[b, h0+h', :, :, :] (one contiguous 331KB HBM range).
                    # HWDGE only (sync+scalar): SWDGE splits runs into ~4KB
                    # packets that waste engine time; HWDGE moves 18.4KB/desc.
                    dma_engs = [
                        nc.sync, nc.scalar, nc.sync, nc.scalar, nc.sync, nc.scalar
                    ]
                    for hp in range(NH):
                        src = bass.AP(
                            z, hp * HW20 * ZROW, [[ZROW, OHW], [1, ZROW]]
                        )
                        dst = bass.AP(
                            out_d,
                            b * OS_B + (h0 + hp) * OS_H,
                            [[ZROW, OHW], [1, ZROW]],
                        )
                        dma_engs[hp].dma_start(dst, src)
                    t_idx += 1
    nc.finalize()
    return nc


_NC_CACHE = None


def _get_nc():
    global _NC_CACHE
    if _NC_CACHE is None:
        _NC_CACHE = _build_nc()
    return _NC_CACHE


def _np_in_dt():
    if _in_dt() == F32:
        return np.float32
    import ml_dtypes

    return ml_dtypes.bfloat16


def _prep_pj(shard: np.ndarray) -> np.ndarray:
    # shard: (4, 20, 20, 32, 4, 4) -> [j, (mnA, i), (b, ph, pw)] padded
    a = shard.reshape(B_LOCAL, HW20, HW20, NI, 4, 4)  # b, ph, pw, i, j(=m), mnA(=n)
    a = a.transpose(4, 5, 3, 0, 1, 2)  # j, mnA, i, b, ph, pw
    a = a.reshape(4, 128, B_LOCAL * HW20 * HW20)
    out = np.zeros((4, 128, PJ_F), dtype=np.float32)
    out[:, :, : B_LOCAL * HW20 * HW20] = a
    return np.ascontiguousarray(out).astype(_np_in_dt())


def _prep_ww(kern: np.ndarray) -> np.ndarray:
    # kern: (3, 3, 32, 32, 4, 4) -> [(mnA, i), (xy, j, o, mnB)] block-diagonal
    k9 = kern.reshape(NXY, NI, NO, 4, 4)  # xy, i, o, j(=m), mnB(=n)
    ww = np.zeros((4, NI, NXY, 4, NO, 4), dtype=np.float32)  # mnA,i,xy,j,o,mnB
    for d in range(4):
        ww[d, :, :, :, :, d] = k9[:, :, :, :, d].transpose(1, 0, 3, 2)  # i,xy,j,o
    return np.ascontiguousarray(ww.reshape(128, NXY * 4 * 128)).astype(_np_in_dt())


LAST_RESULTS = None  # set when CONVCAPS_TRACE=1, for test harness introspection


def kernel(**inputs) -> np.ndarray:
    from concourse.bass_utils import run_bass_kernel_spmd

    poses = np.asarray(inputs["poses"], dtype=np.float32)
    kern = np.asarray(inputs["kernel"], dtype=np.float32)

    nc = _get_nc()
    ww_t = _prep_ww(kern)
    in_maps = []
    for c in range(N_CORES):
        shard = poses[c * B_LOCAL : (c + 1) * B_LOCAL]
        in_maps.append({"pj_t": _prep_pj(shard), "ww_t": ww_t})

    trace = os.environ.get("CONVCAPS_TRACE", "0") == "1"
    res = run_bass_kernel_spmd(
        nc, in_maps, core_ids=list(range(N_CORES)), trace=trace
    )
    if trace:
        global LAST_RESULTS
        LAST_RESULTS = res

    out = np.concatenate(
        [
            r["out"].reshape(B_LOCAL, OHW, OHW, 3, 3, NO, 4, 4)
            for r in res.results
        ],
        axis=0,
    )
    return out


# revision 15
# speedup vs baseline: 1.3168x; 1.1253x over previous
"""ConvCaps (nn_ConvCaps_34995393528409) Trainium2 Bass kernel, v2.

Math: out[b,h,w,x,y,o,m,n] = sum_i poses[b,h+x,w+y,i,m,n] * kernel[x,y,i,o,m,n]

Strategy ("position-major"):
  PSUM/output partition = flat output position c = (h-h0)*20 + w for an
  (b, h-band) tile, so the staging buffer z[c, (xy,o,m,n)] is HBM-ordered and
  the final DMA writes 18KB-contiguous runs (vs 2KB scatter in v1), engaging
  all 16 DMA engines near line rate.

  Per (b, h-band, xy): one PSUM bank holds the complete output-ordered 512-f32
  block (o, m, n). It is filled by 4 matmuls (one per m=j quartet):
    K = (n-member mnA, i) = 128 (block-diagonal weights: W''[(mnA,i),(o,mnB)]
        = kernel[x,y,i,o,(j,mnB)] * delta(mnA,mnB))
    lhsT (stationary) = poses patch [128, M=128 flat positions] -- the (x,y)
        shift is a pure AP offset into PJ[(mnA,i), (b,ph,pw)]
    N = 128 cols (o, mnB) written strided [[16,32],[1,4]] at offset j*4 so the
        bank accumulates (o, m, n) in final order.
  Evacuation is then a contiguous [128, 512] copy per bank (DVE/ACT at full
  rate instead of stride-16 interleave), and per tile one dma_start moves
  z -> out with 108 descriptors of 18.4KB.

  bf16 inputs (fp32 accumulate) halve input DMA and enable FWL weight loads;
  CONVCAPS_IN_DT=f32 falls back to fp32 inputs.
"""

import os

import numpy as np

import concourse.bass as bass
import concourse.tile as tile
from concourse import bacc, mybir
from concourse.vector_clock import ScopedClock

F32 = mybir.dt.float32
BF16 = mybir.dt.bfloat16

N_CORES = 8
B_LOCAL = 4  # 32 / 8
HW20 = 20
OHW = 18
NI = 32
NO = 32
NXY = 9
NH = 6  # h rows per tile
N_HTILES = OHW // NH  # 3
NPOS = 128  # flat positions (partitions) per tile: c = h'*20 + pw, real c%20<18
PJ_F = 1632  # padded free size per j block: >= 3*400 + 14*20 + 2 + 128
BLK = 512  # (o, m, n) floats per (position, xy)
ZROW = NXY * BLK  # 4608 floats per position

# output strides (elements) for the per-core output [4,18,18,9,512]
OS_B = OHW * OHW * NXY * BLK  # 1492992
OS_H = OHW * NXY * BLK  # 82944
OS_W = NXY * BLK  # 4608


def _patch_tile_drain():
    """This walrus build rejects >1 sync-wait on the Tile kernel-tail Drain;
    split the waits across a chain of drains."""
    if getattr(tile.TileContext, "_convcaps_drain_patch", False):
        return

    def _drain_and_barrier(self, tick_clock, wait_clock):
        drain_inst = self.nc.sync.drain()
        wait_clock.add_sem_waits(
            drain_inst.ins, ScopedClock({None: tick_clock.global_clock})
        )
        si = drain_inst.ins.sync_info
        w = list(si.on_wait or []) if si is not None else []
        if len(w) > 1:
            drain_inst.ins.sync_info = mybir.SyncInfo(
                on_wait=w[:1], on_update=list(si.on_update or [])
            )
            for x in w[1:]:
                extra = self.nc.sync.drain()
                extra.ins.sync_info = mybir.SyncInfo(on_wait=[x], on_update=[])
        self.nc.all_engine_barrier()
        assert self.sems is not None
        popped = self.nc._tile_sem_poison_stack.pop()
        assert popped is self._sem_poison
        self.nc.clear_and_free_semaphores(list(self.sems.allocated().values()))
        self.nc.all_engine_barrier()

    tile.TileContext._drain_and_barrier = _drain_and_barrier
    tile.TileContext._convcaps_drain_patch = True


def _in_dt():
    return F32 if os.environ.get("CONVCAPS_IN_DT", "bf16") == "f32" else BF16


def _build_nc():
    _patch_tile_drain()
    IN_DT = _in_dt()
    nc = bacc.Bacc("TRN2", target_bir_lowering=False, num_devices=N_CORES)

    # [j, (mnA, i), (b, ph, pw)] zero-padded to PJ_F
    pj_d = nc.declare_dram_parameter("pj_t", [4, 128, PJ_F], IN_DT, isOutput=False)
    # [(mnA, i), (xy, j, o, mnB)]
    ww_d = nc.declare_dram_parameter("ww_t", [128, NXY * 4 * 128], IN_DT, isOutput=False)
    out_d = nc.declare_dram_parameter(
        "out", [B_LOCAL, OHW, OHW, NXY, BLK], F32, isOutput=True
    )

    pj_sb = nc.alloc_sbuf_tensor("pj_sb", [128, 4 * PJ_F], IN_DT)
    ww_sb = nc.alloc_sbuf_tensor("ww_sb", [128, NXY * 4 * 128], IN_DT)
    z_sb = [nc.alloc_sbuf_tensor(f"z{i}", [128, ZROW], F32) for i in range(6)]

    with tile.TileContext(nc) as tc:
        nc.sync.dma_start(ww_sb.ap()[0:128, :], ww_d.ap()[0:128, :])
        for j in range(4):
            eng = nc.sync if j % 2 == 0 else nc.scalar
            eng.dma_start(
                pj_sb.ap()[0:128, j * PJ_F : (j + 1) * PJ_F], pj_d.ap()[j]
            )

        with tc.tile_pool(name="psum", bufs=8, space="PSUM") as pp:
            t_idx = 0
            for b in range(B_LOCAL):
                for ht in range(N_HTILES):
                    h0 = ht * NH
                    z = z_sb[t_idx % 6]
                    for xy in range(NXY):
                        x, y = xy // 3, xy % 3
                        ps = pp.tile([128, BLK], F32, name="ps", tag="ps")
                        for j in range(4):
                            base = j * PJ_F + b * 400 + (h0 + x) * HW20 + y
                            dst = bass.AP(
                                ps.tensor,
                                j * 4,
                                [[BLK, NPOS], [16, NO], [1, 4]],
                            )
                            nc.tensor.matmul(
                                dst,
                                pj_sb.ap()[0:128, base : base + 128],
                                ww_sb.ap()[
                                    0:128, xy * 512 + j * 128 : xy * 512 + (j + 1) * 128
                                ],
                                start=(j == 0),
                                stop=(j == 3),
                                skip_group_check=True,
                            )
                        zdst = z.ap()[0:NPOS, xy * BLK : (xy + 1) * BLK]
                        if xy % 2 == 0:
                            nc.vector.tensor_copy(zdst, ps[0:NPOS, 0:BLK])
                        else:
                            nc.scalar.copy(zdst, ps[0:NPOS, 0:BLK])
                    # per h row: z[c=h'*20+w (18 partitions), 4608] ->
                    # out[b, h0+h', :, :, :] (one contiguous 331KB HBM range).
                    # HWDGE only (sync+scalar): SWDGE splits runs into ~4KB
                    # packets that waste engine time; HWDGE moves 18.4KB/desc.
                    dma_engs = [
                        nc.sync, nc.scalar, nc.sync, nc.scalar, nc.sync, nc.scalar
                    ]
                    for hp in range(NH):
                        src = bass.AP(
                            z, hp * HW20 * ZROW, [[ZROW, OHW], [1, ZROW]]
                        )
                        dst = bass.AP(
                            out_d,
                            b * OS_B + (h0 + hp) * OS_H,
                            [[ZROW, OHW], [1, ZROW]],
                        )
                        dma_engs[hp].dma_start(dst, src)
                    t_idx += 1
    nc.finalize()
    return nc


_NC_CACHE = None


def _get_nc():
    global _NC_CACHE
    if _NC_CACHE is None:
        _NC_CACHE = _build_nc()
    return _NC_CACHE


def _np_in_dt():
    if _in_dt() == F32:
        return np.float32
    import ml_dtypes

    return ml_dtypes.bfloat16


def _prep_pj(shard: np.ndarray) -> np.ndarray:
    # shard: (4, 20, 20, 32, 4, 4) -> [j, (mnA, i), (b, ph, pw)] padded
    a = shard.reshape(B_LOCAL, HW20, HW20, NI, 4, 4)  # b, ph, pw, i, j(=m), mnA(=n)
    a = a.transpose(4, 5, 3, 0, 1, 2)  # j, mnA, i, b, ph, pw
    a = a.reshape(4, 128, B_LOCAL * HW20 * HW20)
    out = np.zeros((4, 128, PJ_F), dtype=np.float32)
    out[:, :, : B_LOCAL * HW20 * HW20] = a
    return np.ascontiguousarray(out).astype(_np_in_dt())


def _prep_ww(kern: np.ndarray) -> np.ndarray:
    # kern: (3, 3, 32, 32, 4, 4) -> [(mnA, i), (xy, j, o, mnB)] block-diagonal
    k9 = kern.reshape(NXY, NI, NO, 4, 4)  # xy, i, o, j(=m), mnB(=n)
    ww = np.zeros((4, NI, NXY, 4, NO, 4), dtype=np.float32)  # mnA,i,xy,j,o,mnB
    for d in range(4):
        ww[d, :, :, :, :, d] = k9[:, :, :, :, d].transpose(1, 0, 3, 2)  # i,xy,j,o
    return np.ascontiguousarray(ww.reshape(128, NXY * 4 * 128)).astype(_np_in_dt())


LAST_RESULTS = None  # set when CONVCAPS_TRACE=1, for test harness introspection


def kernel(**inputs) -> np.ndarray:
    from concourse.bass_utils import run_bass_kernel_spmd

    poses = np.asarray(inputs["poses"], dtype=np.float32)
    kern = np.asarray(inputs["kernel"], dtype=np.float32)

    nc = _get_nc()
    ww_t = _prep_ww(kern)
    in_maps = []
    for c in range(N_CORES):
        shard = poses[c * B_LOCAL : (c + 1) * B_LOCAL]
        in_maps.append({"pj_t": _prep_pj(shard), "ww_t": ww_t})

    trace = os.environ.get("CONVCAPS_TRACE", "0") == "1"
    res = run_bass_kernel_spmd(
        nc, in_maps, core_ids=list(range(N_CORES)), trace=trace
    )
    if trace:
        global LAST_RESULTS
        LAST_RESULTS = res

    out = np.concatenate(
        [
            r["out"].reshape(B_LOCAL, OHW, OHW, 3, 3, NO, 4, 4)
            for r in res.results
        ],
        axis=0,
    )
    return out
